# revision 7
# baseline (speedup 1.0000x reference)
"""Trainium2 Bass kernel for CNN+GCN+MLP (nn_CNNGCN_18236431139458).

Strategy (8 NeuronCores, one chip):
  - Conv + both GCN layers: data-parallel over batch (4 samples/core).
    The scatter-aggregate is a dense matmul against the normalized
    adjacency A^T (built host-side from edge_index), resident in SBUF
    as fp8 and shared by BOTH GCN layers' aggregations, which run as
    fp8 DoubleRow (2x PE rate). fp8 operands that would land in the
    e4m3 subnormal range are pre-scaled by powers of 2 (x*8, wc*16,
    hw1*8, hw2*32) and the scale is folded back in the activations.
    The conv also runs fp8 DoubleRow: KS padded 3->4 and x shipped as
    two shift-planes so each k-pair is one 256-deep DR contraction.
  - MLP: W1 (262144 x 100) is sharded over rows (nodes) across cores
    and fully resident in SBUF. An on-device AllToAll reshards the GCN
    output from batch-sharded to node-sharded (one batched DMA per
    sample each way); each core computes a partial [32, 100] with its
    W1 shard; a ReduceScatter sums partials and hands each core its
    own 4 samples for the tiny MLP tail.

Layouts (per core):
  xT8  [128 ic, 4 s, 2 j, 2052]  fp8 shift-planes: [.., j, c] = x[c+j]
  wc4  [128 ic, 4 k, 128 oc]     fp8 conv weights *16 (k=3 zero)
  at8  [128 p, 16 c, 2048 dst]   fp8 A^T resident, = A^T[c*128+p, dst]
  h*T  [128 f, 4 s, 2048 n]      bf16 feature-major activations
  hw*  [128 n, 16 nch, 4*128]    fp8 node-major GCN linear outs (scaled)
  h2a  [128 f, 8 i, 4 s, 256 n]  bf16 resharded GCN output
  w1s  [128 f, 256 n, 100]       bf16 W1 row-shard, fully resident
"""

import numpy as np
import ml_dtypes

import concourse.bass as bass
import concourse.mybir as mybir
import concourse.tile as tile
from concourse.tile import add_dep_helper
from concourse import bacc
from concourse.bass_utils import run_bass_kernel_spmd

BF16 = mybir.dt.bfloat16
FP8 = mybir.dt.float8e4
F32 = mybir.dt.float32
NP_BF16 = ml_dtypes.bfloat16
NP_FP8 = mybir.dt.np(FP8)

B, H, E = 32, 2050, 128
HP = 2052                 # padded conv input width (KS 3 -> 4)
N = 2048
C = 128
G1 = G2 = 128
MLPD = 100
KS = 3
NE = 32768
NCORES = 8
BL = B // NCORES          # 4 samples per core
NSH = N // NCORES         # 256 nodes per core (W1 row shard)
RG = [list(range(NCORES))]

# fp8 range scales (undone in the consuming activation)
SX = 8.0                  # x
SW = 16.0                 # conv weights
S1 = 1.0                  # hw1
S2 = 64.0                 # hw2

Relu = mybir.ActivationFunctionType.Relu
Copy = mybir.ActivationFunctionType.Copy
DoubleRow = mybir.MatmulPerfMode.DoubleRow


def _emit_front(nc, tc, pools, tensors):
    """conv + GCN1 + GCN2 -> per-sample h2T [128 f, 8 j, 256 n] bf16.

    lin1 is interleaved into the conv nt-loop and lin2 (+ the hw2 fp8
    residual prep) into the agg1 dt-loop so downstream phases never wait
    on a full linear pass. agg2 runs a 16-pair DR chain: 8 pairs of
    hw2_hi and 8 pairs of the quantization residual res1 = hw2*S2 -
    fp8(hw2*S2), both against the same dithered A^T — recovering most of
    the fp8 precision loss at 2x the matmul cost (still 2x faster than
    bf16).
    """
    acts, psum = pools["acts"], pools["psum"]
    xT_sb = tensors["xT_sb"]
    wc_sb = tensors["wc_sb"]
    cb_sb = tensors["cb_sb"]
    at8_sb = tensors["at8_sb"]

    # ---- conv (fp8 DR) + interleaved lin1 ----
    h0T = acts.tile([128, BL, N], BF16, tag="hT", bufs=2, name="h0T")
    hw1 = acts.tile([128, 16, BL * 128], FP8, tag="hw8", bufs=1, name="hw1")
    for nt in range(4):
        for s in range(BL):
            ps = psum.tile([128, 512], F32, tag="ps", name="ps_conv")
            for kp in range(2):
                nc.tensor.matmul(
                    ps[:],
                    lhsT=wc_sb[:, 2 * kp : 2 * kp + 2, :],
                    rhs=xT_sb[:, s, :, nt * 512 + 2 * kp : nt * 512 + 2 * kp + 512],
                    start=(kp == 0),
                    stop=(kp == 1),
                    perf_mode=DoubleRow,
                )
            act = nc.scalar.activation(h0T[:, s, nt * 512 : (nt + 1) * 512],
                                       ps[:], Relu, bias=cb_sb[:],
                                       scale=1.0 / (SX * SW))
            if s == 0 and nt == 0:
                tensors["anchor_conv0"] = act
        for nch in range(4 * nt, 4 * nt + 4):
            ps = psum.tile([128, 512], F32, tag="ps", name="ps_lin1")
            for s in range(BL):
                nc.tensor.matmul(
                    ps[:, s * 128 : (s + 1) * 128],
                    lhsT=h0T[:, s, nch * 128 : (nch + 1) * 128],
                    rhs=tensors["gw1_sb"][:],
                    start=True,
                    stop=True,
                )
            nc.scalar.activation(hw1[:, nch, :], ps[:], Copy, scale=S1)

    # ---- agg1 (fp8 DR) + interleaved lin2 + hw2 residual prep ----
    h1T = acts.tile([128, BL, N], BF16, tag="hT", bufs=2, name="h1T")
    hw2 = acts.tile([128, 16, BL * 128], FP8, tag="hw8b", bufs=1, name="hw2")
    res1 = acts.tile([128, 16, BL * 128], FP8, tag="res8", bufs=1, name="res1")
    for dt in range(4):
        pss = [psum.tile([128, 512], F32, tag="ps", name=f"ps_agg{s}")
               for s in range(BL)]
        for q in range(8):
            for s in range(BL):
                nc.tensor.matmul(
                    pss[s][:],
                    lhsT=hw1[:, 2 * q : 2 * q + 2, s * 128 : (s + 1) * 128],
                    rhs=at8_sb[:, 2 * q : 2 * q + 2, dt * 512 : (dt + 1) * 512],
                    start=(q == 0),
                    stop=(q == 7),
                    perf_mode=DoubleRow,
                )
        for s in range(BL):
            act = nc.scalar.activation(h1T[:, s, dt * 512 : (dt + 1) * 512],
                                       pss[s][:], Relu, bias=tensors["gb1_sb"][:],
                                       scale=1.0 / S1)
            tensors["anchor_agg1_end"] = act
        tmp32 = acts.tile([128, 4, 512], BF16, tag="tmp32", bufs=2, name="tmp32")
        for nch in range(4 * dt, 4 * dt + 4):
            ps = psum.tile([128, 512], F32, tag="ps", name="ps_lin2")
            for s in range(BL):
                nc.tensor.matmul(
                    ps[:, s * 128 : (s + 1) * 128],
                    lhsT=h1T[:, s, nch * 128 : (nch + 1) * 128],
                    rhs=tensors["gw2_sb"][:],
                    start=True,
                    stop=True,
                )
            nc.scalar.activation(hw2[:, nch, :], ps[:], Copy, scale=S2)
            nc.scalar.activation(tmp32[:, nch - 4 * dt, :], ps[:], Copy, scale=S2)
        nc.vector.tensor_sub(res1[:, 4 * dt : 4 * dt + 4, :],
                             tmp32[:], hw2[:, 4 * dt : 4 * dt + 4, :])

    # ---- agg2: 16-pair fp8 DR chain (hw2_hi then residual), sample-outer
    #      so each sample's h2T finishes early for the A2A ----
    h2Ts = []
    for s in range(BL):
        h2T_s = acts.tile([128, NCORES, NSH], BF16, tag=f"h2T{s}", name=f"h2T{s}")
        pss = [psum.tile([128, 512], F32, tag="ps", name=f"ps_agg2_{dt}")
               for dt in range(4)]
        for q in range(16):
            lhs = (hw2 if q < 8 else res1)
            qq = q % 8
            for dt in range(4):
                nc.tensor.matmul(
                    pss[dt][:],
                    lhsT=lhs[:, 2 * qq : 2 * qq + 2, s * 128 : (s + 1) * 128],
                    rhs=at8_sb[:, 2 * qq : 2 * qq + 2, dt * 512 : (dt + 1) * 512],
                    start=(q == 0),
                    stop=(q == 15),
                    perf_mode=DoubleRow,
                )
        for dt in range(4):
            nc.scalar.activation(h2T_s[:, 2 * dt : 2 * dt + 2, :],
                                 pss[dt][:], Relu, bias=tensors["gb2_sb"][:],
                                 scale=1.0 / S2)
        h2Ts.append(h2T_s)

    return h2Ts


def _emit_tail(nc, tc, pools, tensors, h2Ts, out_ap, collectives=True):
    """AllToAll reshard + sharded MLP + ReduceScatter + local MLP tail."""
    acts, psum, psum2, dram, small = (
        pools["acts"], pools["psum"], pools["psum2"], pools["dram"],
        pools["small"],
    )

    # One AllToAll per local sample, one batched DMA each way.
    h2a = acts.tile([128, NCORES, BL, NSH], BF16, tag="h2a", name="h2a")
    for s in range(BL):
        a2a_in = dram.tile([NCORES, 128, NSH], BF16, tag=f"a2a_in{s}",
                           name=f"a2a_in{s}")
        a2a_out = dram.tile([NCORES, 128, NSH], BF16, tag=f"a2a_out{s}",
                            name=f"a2a_out{s}")
        nc.sync.dma_start(a2a_in.rearrange("j p c -> p j c"), h2Ts[s][:])
        if collectives:
            nc.gpsimd.collective_compute(
                "AllToAll", mybir.AluOpType.bypass, replica_groups=RG,
                ins=[a2a_in.opt()], outs=[a2a_out.opt()],
            )
            nc.sync.dma_start(h2a[:, :, s, :],
                              a2a_out.rearrange("i p c -> p i c"))
        else:
            # timing stand-in: skip the wire, read staged data directly
            nc.sync.dma_start(h2a[:, :, s, :],
                              a2a_in.rearrange("i p c -> p i c"))

    # PE warmers: keep the tensor engine busy across the A2A wait so the
    # MLP matmuls run at full (ramped) clock.
    n_warm = tensors.get("n_warm", 0)
    if n_warm > 0:
        warm_ps = psum.tile([128, 512], F32, tag="ps", name="warm_ps")
        for w in range(n_warm):
            nc.tensor.matmul(
                warm_ps[:, 0:128],
                lhsT=tensors["gw2_sb"][:],
                rhs=tensors["gw1_sb"][:],
                start=(w == 0),
                stop=(w == n_warm - 1),
            )
        warm_sb = small.tile([128, 1], F32, tag="warm_sb", name="warm_sb")
        nc.vector.tensor_copy(warm_sb[:], warm_ps[:, 0:1])
        warm_dr = dram.tile([128, 1], F32, tag="warm_dr", name="warm_dr")
        nc.sync.dma_start(warm_dr[:], warm_sb[:])

    # z[b, c] = sum_n h2a[:, :, :, n].T @ w1s[:, n, :]  (256 k-tiles).
    # M=32 wastes 3/4 of the PE columns, so column-tile: 4 nodes run
    # concurrently in disjoint 32-col groups (tile_position).
    w1s_sb = tensors["w1s_sb"]
    ps_z = psum2.tile([128, MLPD], F32, tag="psz", name="ps_z")
    for n in range(NSH):
        j = n % 4
        nc.tensor.matmul(
            ps_z[32 * j : 32 * (j + 1), :],
            lhsT=h2a[:, :, :, n],
            rhs=w1s_sb[:, n, :],
            start=(n < 4),
            stop=(n >= NSH - 4),
            tile_position=(0, 32 * j),
        )

    z_sb = small.tile([32, MLPD], F32, tag="z_sb", name="z_sb")
    nc.vector.tensor_copy(z_sb[:], ps_z[0:32, :])
    for j in range(1, 4):
        nc.vector.tensor_add(z_sb[:], z_sb[:], ps_z[32 * j : 32 * (j + 1), :])
    rs_in = dram.tile([32, MLPD], F32, tag="rs_in", name="rs_in")
    rs_out = dram.tile([BL, MLPD], F32, tag="rs_out", name="rs_out")
    nc.sync.dma_start(rs_in[:], z_sb[:])
    zloc = small.tile([BL, MLPD], F32, tag="zloc", name="zloc")
    if collectives:
        nc.gpsimd.collective_compute(
            "ReduceScatter", mybir.AluOpType.add, replica_groups=RG,
            ins=[rs_in.opt()], outs=[rs_out.opt()],
        )
        nc.sync.dma_start(zloc[:], rs_out[:])
    else:
        nc.sync.dma_start(zloc[:], rs_in[0:BL, :])
    hm = small.tile([BL, MLPD], F32, tag="hm", name="hm")
    nc.vector.tensor_add(hm[:], zloc[:], tensors["b1r_sb"][:])
    nc.vector.tensor_scalar_max(hm[:], hm[:], 0.0)
    nc.vector.tensor_mul(hm[:], hm[:], tensors["w2r_sb"][:])
    osb = small.tile([BL, 1], F32, tag="osb", name="osb")
    nc.vector.reduce_sum(osb[:], hm[:], axis=mybir.AxisListType.X)
    nc.vector.tensor_add(osb[:], osb[:], tensors["b2r_sb"][:])
    nc.sync.dma_start(out_ap[:], osb[:])


def build_nc(front_reps=1, tail_reps=1, collectives=True, num_devices=NCORES,
             loop_all_reps=1, n_warm=0):
    """Build + compile the SPMD program. Reps>1 variants are for timing.

    loop_all_reps>1 wraps front+tail in a hardware loop with collectives
    replaced by equal-volume DMA stand-ins (collectives can't sit inside
    control flow) — used to measure whole-kernel steady-state time.
    """
    nc = bacc.Bacc("TRN2", target_bir_lowering=False, debug=False,
                   num_devices=num_devices)

    d_xT = nc.dram_tensor("xT", [BL, 2, 128, HP], FP8, kind="ExternalInput").ap()
    d_at8 = nc.dram_tensor("at8", [16, 128, N], FP8, kind="ExternalInput").ap()
    d_wc = nc.dram_tensor("wc", [4, 128, 128], FP8, kind="ExternalInput").ap()
    d_cb = nc.dram_tensor("cb", [128, 1], F32, kind="ExternalInput").ap()
    d_gw1 = nc.dram_tensor("gw1", [128, 128], BF16, kind="ExternalInput").ap()
    d_gb1 = nc.dram_tensor("gb1", [128, 1], F32, kind="ExternalInput").ap()
    d_gw2 = nc.dram_tensor("gw2", [128, 128], BF16, kind="ExternalInput").ap()
    d_gb2 = nc.dram_tensor("gb2", [128, 1], F32, kind="ExternalInput").ap()
    d_w1s = nc.dram_tensor("w1s", [128, NSH, MLPD], BF16, kind="ExternalInput").ap()
    d_b1r = nc.dram_tensor("b1r", [BL, MLPD], F32, kind="ExternalInput").ap()
    d_w2r = nc.dram_tensor("w2r", [BL, MLPD], F32, kind="ExternalInput").ap()
    d_b2r = nc.dram_tensor("b2r", [BL, 1], F32, kind="ExternalInput").ap()
    d_out = nc.dram_tensor("out", [BL, 1], F32, kind="ExternalOutput").ap()

    with tile.TileContext(nc) as tc:
        with (
            tc.tile_pool(name="const", bufs=1) as const,
            tc.tile_pool(name="acts", bufs=1) as acts,
            tc.tile_pool(name="small", bufs=1) as small,
            tc.tile_pool(name="psum", bufs=7, space="PSUM") as psum,
            tc.tile_pool(name="psum2", bufs=1, space="PSUM") as psum2,
            tc.tile_pool(name="dram", bufs=1, space="DRAM") as dram,
        ):
            pools = dict(const=const, acts=acts, small=small,
                         psum=psum, psum2=psum2, dram=dram)

            # ---- load inputs to SBUF (per-sample x slices so conv starts early)
            xT_sb = const.tile([128, BL, 2, HP], FP8, name="xT_sb")
            for s in range(BL):
                nc.sync.dma_start(xT_sb[:, s, :, :],
                                  d_xT[s].rearrange("j p c -> p j c"))
            wc_sb = const.tile([128, 4, 128], FP8, name="wc_sb")
            nc.sync.dma_start(wc_sb[:], d_wc.rearrange("k p o -> p k o"))
            cb_sb = const.tile([128, 1], F32, name="cb_sb")
            nc.sync.dma_start(cb_sb[:], d_cb[:])
            gw1_sb = const.tile([128, 128], BF16, name="gw1_sb")
            nc.sync.dma_start(gw1_sb[:], d_gw1[:])
            gb1_sb = const.tile([128, 1], F32, name="gb1_sb")
            nc.sync.dma_start(gb1_sb[:], d_gb1[:])
            gw2_sb = const.tile([128, 128], BF16, name="gw2_sb")
            nc.sync.dma_start(gw2_sb[:], d_gw2[:])
            gb2_sb = const.tile([128, 1], F32, name="gb2_sb")
            nc.sync.dma_start(gb2_sb[:], d_gb2[:])
            # fp8 A^T resident for BOTH aggregations; chunked load so the
            # first agg1 chunks are ready early
            at8_sb = const.tile([128, 16, N], FP8, name="at8_sb")
            at8_dmas = []
            for q in range(8):
                at8_dmas.append(nc.sync.dma_start(
                    at8_sb[:, 2 * q : 2 * q + 2, :],
                    d_at8[2 * q : 2 * q + 2].rearrange("c p d -> p c d"),
                ))
            # W1 shard fully resident; load starts after A^T
            w1s_sb = const.tile([128, NSH, MLPD], BF16, name="w1s_sb")
            w1_dmas = []
            for ch in range(4):
                w1_dmas.append(nc.sync.dma_start(
                    w1s_sb[:, ch * (NSH // 4) : (ch + 1) * (NSH // 4), :],
                    d_w1s[:, ch * (NSH // 4) : (ch + 1) * (NSH // 4), :],
                ))
            b1r_sb = small.tile([BL, MLPD], F32, name="b1r_sb")
            nc.sync.dma_start(b1r_sb[:], d_b1r[:])
            w2r_sb = small.tile([BL, MLPD], F32, name="w2r_sb")
            nc.sync.dma_start(w2r_sb[:], d_w2r[:])
            b2r_sb = small.tile([BL, 1], F32, name="b2r_sb")
            nc.sync.dma_start(b2r_sb[:], d_b2r[:])

            tensors = dict(
                xT_sb=xT_sb, wc_sb=wc_sb, cb_sb=cb_sb, at8_sb=at8_sb,
                gw1_sb=gw1_sb, gb1_sb=gb1_sb, gw2_sb=gw2_sb, gb2_sb=gb2_sb,
                w1s_sb=w1s_sb, b1r_sb=b1r_sb, w2r_sb=w2r_sb, b2r_sb=b2r_sb,
                n_warm=n_warm,
            )

            if loop_all_reps > 1:
                with tc.For_i(0, loop_all_reps, 1,
                              hint_engines=(mybir.EngineType.PE,)):
                    h2Ts = _emit_front(nc, tc, pools, tensors)
                    _emit_tail(nc, tc, pools, tensors, h2Ts, d_out,
                               collectives=False)
            elif front_reps == 1:
                h2Ts = _emit_front(nc, tc, pools, tensors)
                if "anchor_conv0" in tensors:
                    for d in w1_dmas:
                        add_dep_helper(d.ins, tensors["anchor_conv0"].ins,
                                       reason="delay W1 load past x load + conv start")
                for _ in range(tail_reps):
                    _emit_tail(nc, tc, pools, tensors, h2Ts, d_out,
                               collectives=collectives)
            else:
                with tc.For_i(0, front_reps, 1,
                              hint_engines=(mybir.EngineType.PE,)):
                    h2Ts = _emit_front(nc, tc, pools, tensors)
                for _ in range(tail_reps):
                    _emit_tail(nc, tc, pools, tensors, h2Ts, d_out,
                               collectives=collectives)

    nc.compile()
    return nc


def _dither_fp8(AT):
    """fp8-quantize A^T with per-dst-column error diffusion along src: the
    running quantization error feeds the next nonzero of the column, so each
    column's error sum stays ~0 and the (coherent) aggregate bias vanishes."""
    n = AT.shape[1]
    nzd, nzs = np.nonzero(AT.T)          # sorted by (dst, src)
    counts = np.bincount(nzd, minlength=n)
    kmax = int(counts.max())
    idx = np.full((kmax, n), -1, np.int64)
    pos = np.zeros(n, np.int64)
    for s_, d_ in zip(nzs, nzd):
        idx[pos[d_], d_] = s_
        pos[d_] += 1
    out = np.zeros_like(AT)
    acc = np.zeros(n, np.float32)
    for k in range(kmax):
        cols = np.nonzero(idx[k] >= 0)[0]
        v = AT[idx[k, cols], cols] + acc[cols]
        q = v.astype(NP_FP8).astype(np.float32)
        acc[cols] = v - q
        out[idx[k, cols], cols] = q
    return out


def _prep_inputs(x, edge_index, conv_w, conv_b, gW1, gb1, gW2, gb2, W1, b1, W2, b2):
    """Host-side sharding / layout prep -> per-core input maps."""
    # gcn_norm (add_self_loops=True), duplicated edges accumulate
    src = np.concatenate([np.asarray(edge_index[0]), np.arange(N, dtype=np.int64)])
    dst = np.concatenate([np.asarray(edge_index[1]), np.arange(N, dtype=np.int64)])
    deg = np.bincount(dst, minlength=N).astype(np.float32)
    dinv = (1.0 / np.sqrt(np.maximum(deg, 1.0))).astype(np.float32)
    normv = dinv[src] * dinv[dst]
    AT = np.zeros((N, N), np.float32)
    np.add.at(AT, (src, dst), normv)
    at8_tiled = np.ascontiguousarray(
        _dither_fp8(AT).reshape(16, 128, N)).astype(NP_FP8)

    # conv weights *SW, padded to 4 taps (last = 0), [k, ic, oc]
    wc = np.zeros((4, 128, 128), np.float32)
    wc[:KS] = np.asarray(conv_w)[:, 0, :, :].transpose(1, 2, 0)  # [k, ic, oc]
    wc = np.ascontiguousarray(wc * SW).astype(NP_FP8)
    cb = np.asarray(conv_b, np.float32).reshape(128, 1)
    gw1 = np.asarray(gW1).astype(NP_BF16)
    gb1_ = np.asarray(gb1, np.float32).reshape(128, 1)
    gw2 = np.asarray(gW2).astype(NP_BF16)
    gb2_ = np.asarray(gb2, np.float32).reshape(128, 1)
    b1r = np.ascontiguousarray(np.broadcast_to(np.asarray(b1, np.float32), (BL, MLPD)))
    w2r = np.ascontiguousarray(np.broadcast_to(np.asarray(W2, np.float32)[:, 0], (BL, MLPD)))
    b2r = np.full((BL, 1), np.asarray(b2, np.float32)[0], np.float32)

    W1r = np.asarray(W1, np.float32).reshape(N, G2, MLPD)
    x_np = np.asarray(x, np.float32)
    # shift-planes: xp[b, j, ic, c] = x[b, c+j, ic] * SX, zero-padded
    xpad = np.zeros((B, HP + 1, E), np.float32)
    xpad[:, :H, :] = x_np * SX
    xsh = np.stack([xpad[:, 0:HP, :], xpad[:, 1:HP + 1, :]], axis=1)  # [B,2,HP,E]
    xsh = np.ascontiguousarray(xsh.transpose(0, 1, 3, 2)).astype(NP_FP8)

    in_maps = []
    for c in range(NCORES):
        w1s = np.ascontiguousarray(
            W1r[c * NSH : (c + 1) * NSH].transpose(1, 0, 2)
        ).astype(NP_BF16)
        in_maps.append({
            "xT": xsh[c * BL : (c + 1) * BL], "at8": at8_tiled, "wc": wc,
            "cb": cb, "gw1": gw1, "gb1": gb1_, "gw2": gw2, "gb2": gb2_,
            "w1s": w1s, "b1r": b1r, "w2r": w2r, "b2r": b2r,
        })
    return in_maps


_NC_CACHE = {}


def kernel(**inputs) -> np.ndarray:
    key = "full"
    if key not in _NC_CACHE:
        _NC_CACHE[key] = build_nc()
    nc = _NC_CACHE[key]
    in_maps = _prep_inputs(**inputs)
    res = run_bass_kernel_spmd(nc, in_maps, core_ids=list(range(NCORES)))
    out = np.concatenate([res.results[c]["out"] for c in range(NCORES)], axis=0)
    return out.astype(np.float32)


# revision 10
# speedup vs baseline: 1.4782x; 1.4782x over previous
"""Trainium2 Bass kernel for CNN+GCN+MLP (nn_CNNGCN_18236431139458).

Strategy (8 NeuronCores, one chip):
  - Conv + both GCN layers: data-parallel over batch (4 samples/core).
    The scatter-aggregate is a dense matmul against the normalized
    adjacency A^T (built host-side from edge_index), resident in SBUF
    as fp8 and shared by BOTH GCN layers' aggregations, which run as
    fp8 DoubleRow (2x PE rate). fp8 operands that would land in the
    e4m3 subnormal range are pre-scaled by powers of 2 (x*8, wc*16,
    hw1*8, hw2*32) and the scale is folded back in the activations.
    The conv also runs fp8 DoubleRow: KS padded 3->4 and x shipped as
    two shift-planes so each k-pair is one 256-deep DR contraction.
  - MLP: W1 (262144 x 100) is sharded over rows (nodes) across cores
    and fully resident in SBUF. An on-device AllToAll reshards the GCN
    output from batch-sharded to node-sharded (one batched DMA per
    sample each way); each core computes a partial [32, 100] with its
    W1 shard; a ReduceScatter sums partials and hands each core its
    own 4 samples for the tiny MLP tail.

Layouts (per core):
  xT8  [128 ic, 4 s, 2 j, 2052]  fp8 shift-planes: [.., j, c] = x[c+j]
  wc4  [128 ic, 4 k, 128 oc]     fp8 conv weights *16 (k=3 zero)
  at8  [128 p, 16 c, 2048 dst]   fp8 A^T resident, = A^T[c*128+p, dst]
  h*T  [128 f, 4 s, 2048 n]      bf16 feature-major activations
  hw*  [128 n, 16 nch, 4*128]    fp8 node-major GCN linear outs (scaled)
  h2a  [128 f, 8 i, 4 s, 256 n]  bf16 resharded GCN output
  w1s  [128 f, 256 n, 100]       bf16 W1 row-shard, fully resident
"""

import numpy as np
import ml_dtypes

import concourse.bass as bass
import concourse.mybir as mybir
import concourse.tile as tile
from concourse.tile import add_dep_helper
from concourse import bacc
from concourse.bass_utils import run_bass_kernel_spmd

BF16 = mybir.dt.bfloat16
FP8 = mybir.dt.float8e4
F32 = mybir.dt.float32
NP_BF16 = ml_dtypes.bfloat16
NP_FP8 = mybir.dt.np(FP8)

B, H, E = 32, 2050, 128
HP = 2052                 # padded conv input width (KS 3 -> 4)
N = 2048
C = 128
G1 = G2 = 128
MLPD = 100
KS = 3
NE = 32768
NCORES = 8
BL = B // NCORES          # 4 samples per core
NSH = N // NCORES         # 256 nodes per core (W1 row shard)
RG = [list(range(NCORES))]

# fp8 range scales (undone in the consuming activation)
SX = 8.0                  # x
SW = 16.0                 # conv weights
S1 = 1.0                  # hw1
S2 = 64.0                 # hw2

Relu = mybir.ActivationFunctionType.Relu
Copy = mybir.ActivationFunctionType.Copy
DoubleRow = mybir.MatmulPerfMode.DoubleRow


def _emit_front(nc, tc, pools, tensors):
    """conv + GCN1 + GCN2 -> per-sample h2T [128 f, 8 j, 256 n] bf16.

    lin1 is interleaved into the conv nt-loop and lin2 (+ the hw2 fp8
    residual prep) into the agg1 dt-loop so downstream phases never wait
    on a full linear pass. agg2 runs a 16-pair DR chain: 8 pairs of
    hw2_hi and 8 pairs of the quantization residual res1 = hw2*S2 -
    fp8(hw2*S2), both against the same dithered A^T — recovering most of
    the fp8 precision loss at 2x the matmul cost (still 2x faster than
    bf16).
    """
    acts, psum = pools["acts"], pools["psum"]
    xT_sb = tensors["xT_sb"]
    wc_sb = tensors["wc_sb"]
    cb_sb = tensors["cb_sb"]
    at8_sb = tensors["at8_sb"]

    # ---- conv (fp8 DR) + interleaved lin1 ----
    h0T = acts.tile([128, BL, N], BF16, tag="hT", bufs=2, name="h0T")
    hw1 = acts.tile([128, 16, BL * 128], FP8, tag="hw8", bufs=1, name="hw1")
    for nt in range(4):
        for s in range(BL):
            ps = psum.tile([128, 512], F32, tag="ps", name="ps_conv")
            for kp in range(2):
                nc.tensor.matmul(
                    ps[:],
                    lhsT=wc_sb[:, 2 * kp : 2 * kp + 2, :],
                    rhs=xT_sb[:, s, :, nt * 512 + 2 * kp : nt * 512 + 2 * kp + 512],
                    start=(kp == 0),
                    stop=(kp == 1),
                    perf_mode=DoubleRow,
                )
            act = nc.scalar.activation(h0T[:, s, nt * 512 : (nt + 1) * 512],
                                       ps[:], Relu, bias=cb_sb[:],
                                       scale=1.0 / (SX * SW))
            if s == 0 and nt == 0:
                tensors["anchor_conv0"] = act
        for nch in range(4 * nt, 4 * nt + 4):
            ps = psum.tile([128, 512], F32, tag="ps", name="ps_lin1")
            for s in range(BL):
                nc.tensor.matmul(
                    ps[:, s * 128 : (s + 1) * 128],
                    lhsT=h0T[:, s, nch * 128 : (nch + 1) * 128],
                    rhs=tensors["gw1_sb"][:],
                    start=True,
                    stop=True,
                )
            nc.vector.tensor_copy(hw1[:, nch, :], ps[:])

    # ---- agg1 (fp8 DR) + interleaved lin2 + hw2 residual prep ----
    h1T = acts.tile([128, BL, N], BF16, tag="hT", bufs=2, name="h1T")
    hw2 = acts.tile([128, 16, BL * 128], FP8, tag="hw8b", bufs=1, name="hw2")
    res1 = acts.tile([128, 16, BL * 128], FP8, tag="res8", bufs=1, name="res1")
    for dt in range(4):
        pss = [psum.tile([128, 512], F32, tag="ps", name=f"ps_agg{s}")
               for s in range(BL)]
        for q in range(8):
            for s in range(BL):
                nc.tensor.matmul(
                    pss[s][:],
                    lhsT=hw1[:, 2 * q : 2 * q + 2, s * 128 : (s + 1) * 128],
                    rhs=at8_sb[:, 2 * q : 2 * q + 2, dt * 512 : (dt + 1) * 512],
                    start=(q == 0),
                    stop=(q == 7),
                    perf_mode=DoubleRow,
                )
        for s in range(BL):
            act = nc.scalar.activation(h1T[:, s, dt * 512 : (dt + 1) * 512],
                                       pss[s][:], Relu, bias=tensors["gb1_sb"][:],
                                       scale=1.0 / S1)
            tensors["anchor_agg1_end"] = act
        # gw2 is host-scaled by S2, so ps already holds hw2*S2: the fp8
        # quantize is a plain copy and the residual a direct psum subtract.
        for nch in range(4 * dt, 4 * dt + 4):
            ps = psum.tile([128, 512], F32, tag="ps", name="ps_lin2")
            for s in range(BL):
                nc.tensor.matmul(
                    ps[:, s * 128 : (s + 1) * 128],
                    lhsT=h1T[:, s, nch * 128 : (nch + 1) * 128],
                    rhs=tensors["gw2_sb"][:],
                    start=True,
                    stop=True,
                )
            nc.vector.tensor_copy(hw2[:, nch, :], ps[:])
            nc.vector.tensor_sub(res1[:, nch, :], ps[:], hw2[:, nch, :])

    # ---- agg2: 16-pair fp8 DR chain (hw2_hi then residual), sample-outer
    #      so each sample's h2T finishes early for the A2A ----
    h2Ts = []
    for s in range(BL):
        h2T_s = acts.tile([128, NCORES, NSH], BF16, tag=f"h2T{s}", name=f"h2T{s}")
        pss = [psum.tile([128, 512], F32, tag="ps", name=f"ps_agg2_{dt}")
               for dt in range(4)]
        for q in range(16):
            lhs = (hw2 if q < 8 else res1)
            qq = q % 8
            for dt in range(4):
                nc.tensor.matmul(
                    pss[dt][:],
                    lhsT=lhs[:, 2 * qq : 2 * qq + 2, s * 128 : (s + 1) * 128],
                    rhs=at8_sb[:, 2 * qq : 2 * qq + 2, dt * 512 : (dt + 1) * 512],
                    start=(q == 0),
                    stop=(q == 15),
                    perf_mode=DoubleRow,
                )
        for dt in range(4):
            nc.scalar.activation(h2T_s[:, 2 * dt : 2 * dt + 2, :],
                                 pss[dt][:], Relu, bias=tensors["gb2_sb"][:],
                                 scale=1.0 / S2)
        h2Ts.append(h2T_s)

    return h2Ts


def _emit_tail(nc, tc, pools, tensors, h2Ts, out_ap, collectives=True):
    """AllToAll reshard + sharded MLP + ReduceScatter + local MLP tail."""
    acts, psum, psum2, dram, small = (
        pools["acts"], pools["psum"], pools["psum2"], pools["dram"],
        pools["small"],
    )

    # One AllToAll per local sample, one batched DMA each way.
    h2a = acts.tile([128, NCORES, BL, NSH], BF16, tag="h2a", name="h2a")
    for s in range(BL):
        a2a_in = dram.tile([NCORES, 128, NSH], BF16, tag=f"a2a_in{s}",
                           name=f"a2a_in{s}")
        a2a_out = dram.tile([NCORES, 128, NSH], BF16, tag=f"a2a_out{s}",
                            name=f"a2a_out{s}")
        nc.sync.dma_start(a2a_in.rearrange("j p c -> p j c"), h2Ts[s][:])
        if collectives:
            nc.gpsimd.collective_compute(
                "AllToAll", mybir.AluOpType.bypass, replica_groups=RG,
                ins=[a2a_in.opt()], outs=[a2a_out.opt()],
            )
            nc.sync.dma_start(h2a[:, :, s, :],
                              a2a_out.rearrange("i p c -> p i c"))
        else:
            # timing stand-in: skip the wire, read staged data directly
            nc.sync.dma_start(h2a[:, :, s, :],
                              a2a_in.rearrange("i p c -> p i c"))

    # PE warmers: keep the tensor engine busy across the A2A wait so the
    # MLP matmuls run at full (ramped) clock.
    n_warm = tensors.get("n_warm", 0)
    if n_warm > 0:
        warm_ps = psum.tile([128, 512], F32, tag="ps", name="warm_ps")
        for w in range(n_warm):
            nc.tensor.matmul(
                warm_ps[:, 0:128],
                lhsT=tensors["gw2_sb"][:],
                rhs=tensors["gw1_sb"][:],
                start=(w == 0),
                stop=(w == n_warm - 1),
            )
        warm_sb = small.tile([128, 1], F32, tag="warm_sb", name="warm_sb")
        nc.vector.tensor_copy(warm_sb[:], warm_ps[:, 0:1])
        warm_dr = dram.tile([128, 1], F32, tag="warm_dr", name="warm_dr")
        nc.sync.dma_start(warm_dr[:], warm_sb[:])

    # z[b, c] = sum_n h2a[:, :, :, n].T @ w1s[:, n, :]  (256 k-tiles).
    # M=32 wastes 3/4 of the PE columns, so column-tile: 4 nodes run
    # concurrently in disjoint 32-col groups (tile_position).
    w1s_sb = tensors["w1s_sb"]
    ps_z = psum2.tile([128, MLPD], F32, tag="psz", name="ps_z")
    for n in range(NSH):
        j = n % 4
        nc.tensor.matmul(
            ps_z[32 * j : 32 * (j + 1), :],
            lhsT=h2a[:, :, :, n],
            rhs=w1s_sb[:, n, :],
            start=(n < 4),
            stop=(n >= NSH - 4),
            tile_position=(0, 32 * j),
        )

    z_sb = small.tile([32, MLPD], F32, tag="z_sb", name="z_sb")
    nc.vector.tensor_copy(z_sb[:], ps_z[0:32, :])
    for j in range(1, 4):
        nc.vector.tensor_add(z_sb[:], z_sb[:], ps_z[32 * j : 32 * (j + 1), :])
    rs_in = dram.tile([32, MLPD], F32, tag="rs_in", name="rs_in")
    rs_out = dram.tile([BL, MLPD], F32, tag="rs_out", name="rs_out")
    nc.sync.dma_start(rs_in[:], z_sb[:])
    zloc = small.tile([BL, MLPD], F32, tag="zloc", name="zloc")
    if collectives:
        nc.gpsimd.collective_compute(
            "ReduceScatter", mybir.AluOpType.add, replica_groups=RG,
            ins=[rs_in.opt()], outs=[rs_out.opt()],
        )
        nc.sync.dma_start(zloc[:], rs_out[:])
    else:
        nc.sync.dma_start(zloc[:], rs_in[0:BL, :])
    hm = small.tile([BL, MLPD], F32, tag="hm", name="hm")
    nc.vector.tensor_add(hm[:], zloc[:], tensors["b1r_sb"][:])
    nc.vector.tensor_scalar_max(hm[:], hm[:], 0.0)
    nc.vector.tensor_mul(hm[:], hm[:], tensors["w2r_sb"][:])
    osb = small.tile([BL, 1], F32, tag="osb", name="osb")
    nc.vector.reduce_sum(osb[:], hm[:], axis=mybir.AxisListType.X)
    nc.vector.tensor_add(osb[:], osb[:], tensors["b2r_sb"][:])
    nc.sync.dma_start(out_ap[:], osb[:])


def build_nc(front_reps=1, tail_reps=1, collectives=True, num_devices=NCORES,
             loop_all_reps=1, n_warm=0):
    """Build + compile the SPMD program. Reps>1 variants are for timing.

    loop_all_reps>1 wraps front+tail in a hardware loop with collectives
    replaced by equal-volume DMA stand-ins (collectives can't sit inside
    control flow) — used to measure whole-kernel steady-state time.
    """
    nc = bacc.Bacc("TRN2", target_bir_lowering=False, debug=False,
                   num_devices=num_devices)

    d_xT = nc.dram_tensor("xT", [BL, 2, 128, HP], FP8, kind="ExternalInput").ap()
    d_at8 = nc.dram_tensor("at8", [16, 128, N], FP8, kind="ExternalInput").ap()
    d_wc = nc.dram_tensor("wc", [4, 128, 128], FP8, kind="ExternalInput").ap()
    d_cb = nc.dram_tensor("cb", [128, 1], F32, kind="ExternalInput").ap()
    d_gw1 = nc.dram_tensor("gw1", [128, 128], BF16, kind="ExternalInput").ap()
    d_gb1 = nc.dram_tensor("gb1", [128, 1], F32, kind="ExternalInput").ap()
    d_gw2 = nc.dram_tensor("gw2", [128, 128], BF16, kind="ExternalInput").ap()
    d_gb2 = nc.dram_tensor("gb2", [128, 1], F32, kind="ExternalInput").ap()
    d_w1s = nc.dram_tensor("w1s", [128, NSH, MLPD], BF16, kind="ExternalInput").ap()
    d_b1r = nc.dram_tensor("b1r", [BL, MLPD], F32, kind="ExternalInput").ap()
    d_w2r = nc.dram_tensor("w2r", [BL, MLPD], F32, kind="ExternalInput").ap()
    d_b2r = nc.dram_tensor("b2r", [BL, 1], F32, kind="ExternalInput").ap()
    d_out = nc.dram_tensor("out", [BL, 1], F32, kind="ExternalOutput").ap()

    with tile.TileContext(nc) as tc:
        with (
            tc.tile_pool(name="const", bufs=1) as const,
            tc.tile_pool(name="acts", bufs=1) as acts,
            tc.tile_pool(name="small", bufs=1) as small,
            tc.tile_pool(name="psum", bufs=7, space="PSUM") as psum,
            tc.tile_pool(name="psum2", bufs=1, space="PSUM") as psum2,
            tc.tile_pool(name="dram", bufs=1, space="DRAM") as dram,
        ):
            pools = dict(const=const, acts=acts, small=small,
                         psum=psum, psum2=psum2, dram=dram)

            # ---- load inputs to SBUF (per-sample x slices so conv starts early)
            xT_sb = const.tile([128, BL, 2, HP], FP8, name="xT_sb")
            for s in range(BL):
                nc.sync.dma_start(xT_sb[:, s, :, :],
                                  d_xT[s].rearrange("j p c -> p j c"))
            wc_sb = const.tile([128, 4, 128], FP8, name="wc_sb")
            nc.sync.dma_start(wc_sb[:], d_wc.rearrange("k p o -> p k o"))
            cb_sb = const.tile([128, 1], F32, name="cb_sb")
            nc.sync.dma_start(cb_sb[:], d_cb[:])
            gw1_sb = const.tile([128, 128], BF16, name="gw1_sb")
            nc.sync.dma_start(gw1_sb[:], d_gw1[:])
            gb1_sb = const.tile([128, 1], F32, name="gb1_sb")
            nc.sync.dma_start(gb1_sb[:], d_gb1[:])
            gw2_sb = const.tile([128, 128], BF16, name="gw2_sb")
            nc.sync.dma_start(gw2_sb[:], d_gw2[:])
            gb2_sb = const.tile([128, 1], F32, name="gb2_sb")
            nc.sync.dma_start(gb2_sb[:], d_gb2[:])
            # fp8 A^T resident for BOTH aggregations; chunked load so the
            # first agg1 chunks are ready early
            at8_sb = const.tile([128, 16, N], FP8, name="at8_sb")
            at8_dmas = []
            for q in range(8):
                at8_dmas.append(nc.sync.dma_start(
                    at8_sb[:, 2 * q : 2 * q + 2, :],
                    d_at8[2 * q : 2 * q + 2].rearrange("c p d -> p c d"),
                ))
            # W1 shard fully resident; load starts after A^T
            w1s_sb = const.tile([128, NSH, MLPD], BF16, name="w1s_sb")
            w1_dmas = []
            for ch in range(4):
                w1_dmas.append(nc.sync.dma_start(
                    w1s_sb[:, ch * (NSH // 4) : (ch + 1) * (NSH // 4), :],
                    d_w1s[:, ch * (NSH // 4) : (ch + 1) * (NSH // 4), :],
                ))
            b1r_sb = small.tile([BL, MLPD], F32, name="b1r_sb")
            nc.sync.dma_start(b1r_sb[:], d_b1r[:])
            w2r_sb = small.tile([BL, MLPD], F32, name="w2r_sb")
            nc.sync.dma_start(w2r_sb[:], d_w2r[:])
            b2r_sb = small.tile([BL, 1], F32, name="b2r_sb")
            nc.sync.dma_start(b2r_sb[:], d_b2r[:])

            tensors = dict(
                xT_sb=xT_sb, wc_sb=wc_sb, cb_sb=cb_sb, at8_sb=at8_sb,
                gw1_sb=gw1_sb, gb1_sb=gb1_sb, gw2_sb=gw2_sb, gb2_sb=gb2_sb,
                w1s_sb=w1s_sb, b1r_sb=b1r_sb, w2r_sb=w2r_sb, b2r_sb=b2r_sb,
                n_warm=n_warm,
            )

            if loop_all_reps > 1:
                with tc.For_i(0, loop_all_reps, 1,
                              hint_engines=(mybir.EngineType.PE,)):
                    h2Ts = _emit_front(nc, tc, pools, tensors)
                    _emit_tail(nc, tc, pools, tensors, h2Ts, d_out,
                               collectives=False)
            elif front_reps == 1:
                h2Ts = _emit_front(nc, tc, pools, tensors)
                if "anchor_conv0" in tensors:
                    for d in w1_dmas:
                        add_dep_helper(d.ins, tensors["anchor_conv0"].ins,
                                       reason="delay W1 load past x load + conv start")
                for _ in range(tail_reps):
                    _emit_tail(nc, tc, pools, tensors, h2Ts, d_out,
                               collectives=collectives)
            else:
                with tc.For_i(0, front_reps, 1,
                              hint_engines=(mybir.EngineType.PE,)):
                    h2Ts = _emit_front(nc, tc, pools, tensors)
                for _ in range(tail_reps):
                    _emit_tail(nc, tc, pools, tensors, h2Ts, d_out,
                               collectives=collectives)

    nc.compile()
    return nc


def _dither_fp8(AT):
    """fp8-quantize A^T with per-dst-column error diffusion along src: the
    running quantization error feeds the next nonzero of the column, so each
    column's error sum stays ~0 and the (coherent) aggregate bias vanishes."""
    n = AT.shape[1]
    nzd, nzs = np.nonzero(AT.T)          # sorted by (dst, src)
    counts = np.bincount(nzd, minlength=n)
    kmax = int(counts.max())
    idx = np.full((kmax, n), -1, np.int64)
    pos = np.zeros(n, np.int64)
    for s_, d_ in zip(nzs, nzd):
        idx[pos[d_], d_] = s_
        pos[d_] += 1
    out = np.zeros_like(AT)
    acc = np.zeros(n, np.float32)
    for k in range(kmax):
        cols = np.nonzero(idx[k] >= 0)[0]
        v = AT[idx[k, cols], cols] + acc[cols]
        q = v.astype(NP_FP8).astype(np.float32)
        acc[cols] = v - q
        out[idx[k, cols], cols] = q
    return out


def _prep_inputs(x, edge_index, conv_w, conv_b, gW1, gb1, gW2, gb2, W1, b1, W2, b2):
    """Host-side sharding / layout prep -> per-core input maps."""
    # gcn_norm (add_self_loops=True), duplicated edges accumulate
    src = np.concatenate([np.asarray(edge_index[0]), np.arange(N, dtype=np.int64)])
    dst = np.concatenate([np.asarray(edge_index[1]), np.arange(N, dtype=np.int64)])
    deg = np.bincount(dst, minlength=N).astype(np.float32)
    dinv = (1.0 / np.sqrt(np.maximum(deg, 1.0))).astype(np.float32)
    normv = dinv[src] * dinv[dst]
    AT = np.zeros((N, N), np.float32)
    np.add.at(AT, (src, dst), normv)
    at8_tiled = np.ascontiguousarray(
        _dither_fp8(AT).reshape(16, 128, N)).astype(NP_FP8)

    # conv weights *SW, padded to 4 taps (last = 0), [k, ic, oc]
    wc = np.zeros((4, 128, 128), np.float32)
    wc[:KS] = np.asarray(conv_w)[:, 0, :, :].transpose(1, 2, 0)  # [k, ic, oc]
    wc = np.ascontiguousarray(wc * SW).astype(NP_FP8)
    cb = np.asarray(conv_b, np.float32).reshape(128, 1)
    gw1 = np.asarray(gW1).astype(NP_BF16)
    gb1_ = np.asarray(gb1, np.float32).reshape(128, 1)
    gw2 = (np.asarray(gW2) * S2).astype(NP_BF16)   # pre-scaled: psum = hw2*S2
    gb2_ = np.asarray(gb2, np.float32).reshape(128, 1)
    b1r = np.ascontiguousarray(np.broadcast_to(np.asarray(b1, np.float32), (BL, MLPD)))
    w2r = np.ascontiguousarray(np.broadcast_to(np.asarray(W2, np.float32)[:, 0], (BL, MLPD)))
    b2r = np.full((BL, 1), np.asarray(b2, np.float32)[0], np.float32)

    W1r = np.asarray(W1, np.float32).reshape(N, G2, MLPD)
    x_np = np.asarray(x, np.float32)
    # shift-planes: xp[b, j, ic, c] = x[b, c+j, ic] * SX, zero-padded
    xpad = np.zeros((B, HP + 1, E), np.float32)
    xpad[:, :H, :] = x_np * SX
    xsh = np.stack([xpad[:, 0:HP, :], xpad[:, 1:HP + 1, :]], axis=1)  # [B,2,HP,E]
    xsh = np.ascontiguousarray(xsh.transpose(0, 1, 3, 2)).astype(NP_FP8)

    in_maps = []
    for c in range(NCORES):
        w1s = np.ascontiguousarray(
            W1r[c * NSH : (c + 1) * NSH].transpose(1, 0, 2)
        ).astype(NP_BF16)
        in_maps.append({
            "xT": xsh[c * BL : (c + 1) * BL], "at8": at8_tiled, "wc": wc,
            "cb": cb, "gw1": gw1, "gb1": gb1_, "gw2": gw2, "gb2": gb2_,
            "w1s": w1s, "b1r": b1r, "w2r": w2r, "b2r": b2r,
        })
    return in_maps


_NC_CACHE = {}


def kernel(**inputs) -> np.ndarray:
    key = "full"
    if key not in _NC_CACHE:
        _NC_CACHE[key] = build_nc()
    nc = _NC_CACHE[key]
    in_maps = _prep_inputs(**inputs)
    res = run_bass_kernel_spmd(nc, in_maps, core_ids=list(range(NCORES)))
    out = np.concatenate([res.results[c]["out"] for c in range(NCORES)], axis=0)
    return out.astype(np.float32)
